# revision 17
# baseline (speedup 1.0000x reference)
"""GCNConv (N=20000, E=320000, D=1024) on 8 trn2 NeuronCores.

out = segment_sum(norm * h[col] -> row) with self-loops, h = x @ W^T + b
    = (segment_sum(norm * x[col] -> row)) @ W^T + s[row] * b,
      s[r] = sum of incoming norm (incl. self), norm = deg^-1/2 outer.

Aggregate-then-transform: since the linear layer commutes with the
segment sum, each core stages the FULL raw x table (bf16) in its local
DRAM and no collective is needed at all.  Nodes are sharded 2500/core
(20 dest blocks of 128).  Per dest block: dma_gather source rows from
the local x table, selection-matmul with the norm weights folded into
the selection matrix (self-loops are ordinary edges with weight
dis[r]^2), accumulate z in fp32 PSUM, PE-transpose z, then
z @ W^T with the bias folded in as a rank-1 9th k-tile.
"""

import numpy as np
import ml_dtypes

import concourse.bacc as bacc
import concourse.mybir as mybir
import concourse.tile as tile
from concourse import bass
from concourse import bass_utils
from concourse.masks import make_identity

N = 20000
E = 320000
D = 1024
NC = 8
NPC = N // NC            # 2500 real dest nodes per core
NBLK = 20                # dest blocks of 128 per core
NPCP = NBLK * 128        # 2560 padded dest nodes per core
P = 128
KT = D // P              # 8 contraction tiles of the out GEMM

_cache = {}


def _preprocess(x, edge_index, W, b):
    x = np.asarray(x, dtype=np.float32)
    ei = np.asarray(edge_index)
    W = np.asarray(W, dtype=np.float32)
    b = np.asarray(b, dtype=np.float32)

    row = ei[0].astype(np.int64)
    col = ei[1].astype(np.int64)
    deg = (np.bincount(row, minlength=N) + 1).astype(np.float32)
    dis = deg ** -0.5

    # real edges only (self loops handled via a contiguous per-core path),
    # weight = dis[row]*dis[col]
    rowA = row
    colA = col
    wA = (dis[rowA] * dis[colA]).astype(np.float32)

    # bias scale s[r] = total incoming weight incl. the dis^2 self loop
    s = (np.bincount(rowA, weights=wA, minlength=N) + dis * dis).astype(np.float32)

    core_of = rowA // NPC
    rl = rowA - core_of * NPC
    blk = rl // P
    dl_all = (rl % P).astype(np.float32)

    # per (core, block) edge lists, sorted by source for DMA locality
    key = core_of * NBLK + blk
    order = np.argsort(key, kind="stable")
    ks = key[order]
    bounds = np.searchsorted(ks, np.arange(NC * NBLK + 1))
    colS, dlS, wS = colA[order], dl_all[order], wA[order]
    seg = {}
    cnt = np.zeros((NC, NBLK), dtype=np.int64)
    for c in range(NC):
        for bk in range(NBLK):
            i0, i1 = bounds[c * NBLK + bk], bounds[c * NBLK + bk + 1]
            o2 = np.argsort(colS[i0:i1], kind="stable")
            seg[(c, bk)] = (colS[i0:i1][o2], dlS[i0:i1][o2], wS[i0:i1][o2])
            cnt[c, bk] = i1 - i0

    # shared (SPMD lockstep) schedule: tiles per block = max over cores
    T_bs = tuple(int(-(-int(cnt[:, bk].max()) // P)) for bk in range(NBLK))
    NT = sum(T_bs)

    dl = np.zeros((NC, NT, P), dtype=ml_dtypes.bfloat16)
    wv = np.zeros((NC, NT, P), dtype=ml_dtypes.bfloat16)
    idx16 = np.zeros((NC, 16, NT * 8), dtype=np.int16)
    t0 = 0
    for bk in range(NBLK):
        Tb = T_bs[bk]
        for c in range(NC):
            cc, dd, ww = seg[(c, bk)]
            n = len(cc)
            dl[c, t0 : t0 + Tb].reshape(-1)[:n] = dd.astype(ml_dtypes.bfloat16)
            wv[c, t0 : t0 + Tb].reshape(-1)[:n] = ww.astype(ml_dtypes.bfloat16)
            buf = np.zeros(Tb * P, dtype=np.int16)
            buf[:n] = cc.astype(np.int16)
            idx16[c, :, t0 * 8 : (t0 + Tb) * 8] = buf.reshape(Tb * 8, 16).T
        t0 += Tb

    # GEMM rhs: 8 k-tiles of W^T; bias applied on vector as (b * s) + out
    WT8 = np.zeros((P, KT * D), dtype=ml_dtypes.bfloat16)
    WTc = W.T.astype(ml_dtypes.bfloat16)
    for k in range(KT):
        WT8[:, k * D : (k + 1) * D] = WTc[k * P : (k + 1) * P, :]
    bbc = np.ascontiguousarray(
        np.broadcast_to(b.astype(ml_dtypes.bfloat16), (P, D))
    )

    xw = np.ascontiguousarray(x.astype(ml_dtypes.bfloat16))  # gather table

    # constant idx table 0..127 for the transposing z gather
    tbuf = np.arange(P, dtype=np.int16)
    tix = np.ascontiguousarray(np.tile(tbuf.reshape(8, 16).T, (8, 1)))  # [128, 8]

    in_maps = []
    for c in range(NC):
        xself = np.zeros((NPCP, D), dtype=ml_dtypes.bfloat16)
        xself[:NPC] = xw[c * NPC : (c + 1) * NPC]
        d2p = np.zeros(NPCP, dtype=np.float32)
        d2p[:NPC] = dis[c * NPC : (c + 1) * NPC] ** 2
        d2 = d2p.reshape(NBLK, P).T  # d2[p, bk] = dis^2 of local node bk*128+p
        sp = np.zeros(NPCP, dtype=np.float32)
        sp[:NPC] = s[c * NPC : (c + 1) * NPC]
        sT = sp.reshape(NBLK, P).T  # sT[p, bk] = s of local node bk*128+p
        in_maps.append(
            {
                "xw": xw,
                "xself": xself,
                "dis2": np.ascontiguousarray(d2),
                "sT": np.ascontiguousarray(sT),
                "WT8": np.ascontiguousarray(WT8),
                "bbc": bbc,
                "tix": tix,
                "idx16": np.ascontiguousarray(np.tile(idx16[c], (8, 1))),
                "dl": np.ascontiguousarray(dl[c].T),  # [128, NT]
                "wv": np.ascontiguousarray(wv[c].T),  # [128, NT]
            }
        )
    return T_bs, NT, in_maps


def _build(T_bs, NT):
    f32 = mybir.dt.float32
    bf16 = mybir.dt.bfloat16
    i16 = mybir.dt.int16
    i32 = mybir.dt.int32
    TMAX = max(T_bs)
    tstart = {}
    t0 = 0
    for bk in range(NBLK):
        tstart[bk] = t0
        t0 += T_bs[bk]

    nc = bacc.Bacc("TRN2", target_bir_lowering=False, debug=False,
                   num_devices=NC, num_swdge_queues=2)
    xw = nc.dram_tensor("xw", [N, D], bf16, kind="ExternalInput").ap()
    xself = nc.dram_tensor("xself", [NPCP, D], bf16, kind="ExternalInput").ap()
    dis2 = nc.dram_tensor("dis2", [P, NBLK], f32, kind="ExternalInput").ap()
    sTi = nc.dram_tensor("sT", [P, NBLK], f32, kind="ExternalInput").ap()
    WT8 = nc.dram_tensor("WT8", [P, KT * D], bf16, kind="ExternalInput").ap()
    bbc = nc.dram_tensor("bbc", [P, D], bf16, kind="ExternalInput").ap()
    tix = nc.dram_tensor("tix", [P, 8], i16, kind="ExternalInput").ap()
    idx16 = nc.dram_tensor("idx16", [P, NT * 8], i16, kind="ExternalInput").ap()
    dl = nc.dram_tensor("dl", [P, NT], bf16, kind="ExternalInput").ap()
    wv = nc.dram_tensor("wv", [P, NT], bf16, kind="ExternalInput").ap()
    yout = nc.dram_tensor("yout", [NPCP, D], bf16, kind="ExternalOutput").ap()

    chunks = [slice(s0, s0 + 512) for s0 in range(0, D, 512)]

    with tile.TileContext(nc) as tc:
        with tc.tile_pool(name="const", bufs=1) as const, \
             tc.tile_pool(name="dram", bufs=3, space="DRAM") as dram:
            wt_sb = const.tile([P, KT * D], bf16, name="wt_sb")
            for k in range(KT):
                nc.sync.dma_start(wt_sb[:, k * D : (k + 1) * D],
                                  WT8[:, k * D : (k + 1) * D])
            bbc_sb = const.tile([P, D], bf16, name="bbc_sb")
            nc.sync.dma_start(bbc_sb[:], bbc[:])
            tix_sb = const.tile([P, 8], i16, name="tix_sb")
            nc.sync.dma_start(tix_sb[:], tix[:])
            ix_sb = const.tile([P, NT * 8], i16, name="ix_sb")
            nc.sync.dma_start(ix_sb[:], idx16[:])
            dl_sb = const.tile([P, NT], bf16, name="dl_sb")
            nc.sync.dma_start(dl_sb[:], dl[:])
            wv_sb = const.tile([P, NT], bf16, name="wv_sb")
            nc.sync.dma_start(wv_sb[:], wv[:])
            d2_sb = const.tile([P, NBLK], f32, name="d2_sb")
            nc.sync.dma_start(d2_sb[:], dis2[:])
            sT_sb = const.tile([P, NBLK], f32, name="sT_sb")
            nc.sync.dma_start(sT_sb[:], sTi[:])

            iota_rep = const.tile([P, TMAX * P], bf16, name="iota_rep")
            with tc.tile_pool(name="tmpi", bufs=1) as tmpp:
                iota_i = tmpp.tile([P, TMAX * P], i32, name="iota_i")
                nc.gpsimd.iota(iota_i[:], pattern=[[0, TMAX], [1, P]],
                               channel_multiplier=0)
                nc.vector.tensor_copy(iota_rep[:], iota_i[:])

            with tc.tile_pool(name="gath", bufs=3) as gp, \
                 tc.tile_pool(name="sel", bufs=3) as selp, \
                 tc.tile_pool(name="zps", bufs=2, space="PSUM") as zps, \
                 tc.tile_pool(name="ops", bufs=2, space="PSUM") as ops, \
                 tc.tile_pool(name="zsb", bufs=3) as zsbp, \
                 tc.tile_pool(name="ztb", bufs=3) as ztbp, \
                 tc.tile_pool(name="xsf", bufs=2) as xsfp, \
                 tc.tile_pool(name="aout", bufs=2) as aoutp:
                GSUB = 5
                gq = 0
                zts = {}

                def gemm_block(bk):
                    # out = z @ W^T, bias on vector: ob = b * s + out
                    zt = zts.pop(bk)
                    op_ = ops.tile([P, D], f32)
                    for k in range(KT):
                        for cs in chunks:
                            nc.tensor.matmul(
                                op_[:, cs], lhsT=zt[:, k, :],
                                rhs=wt_sb[:, k * D + cs.start : k * D + cs.stop],
                                start=(k == 0), stop=(k == KT - 1),
                            )
                    ob = aoutp.tile([P, D], bf16)
                    nc.vector.scalar_tensor_tensor(
                        out=ob[:], in0=bbc_sb[:], scalar=sT_sb[:, bk : bk + 1],
                        in1=op_[:], op0=mybir.AluOpType.mult,
                        op1=mybir.AluOpType.add,
                    )
                    nc.sync.dma_start(yout[bk * P : (bk + 1) * P, :], ob[:])

                for bk in range(NBLK):
                    Tb = T_bs[bk]
                    t0 = tstart[bk]
                    g = gp.tile([P, Tb, D], bf16, tag="g")
                    xs = xsfp.tile([P, D], bf16, tag="xs")
                    nc.sync.dma_start(xs[:], xself[bk * P : (bk + 1) * P, :])
                    for s0 in range(0, Tb, GSUB):
                        sn = min(GSUB, Tb - s0)
                        nc.gpsimd.dma_gather(
                            g[:, s0 : s0 + sn, :],
                            xw[:],
                            ix_sb[:, (t0 + s0) * 8 : (t0 + s0 + sn) * 8],
                            sn * P,
                            sn * P,
                            D,
                            queue_num=gq,
                            single_packet=False,
                        )
                        gq = 1 - gq
                    # selT[e, t, d] = (iota[d] == dl[e, t]) * w[e, t]
                    selb = selp.tile([P, Tb, P], bf16, tag="selb")
                    dlb = (dl_sb[:, t0 : t0 + Tb]
                           .rearrange("p (t o) -> p t o", o=1)
                           .to_broadcast([P, Tb, P]))
                    nc.vector.tensor_tensor(
                        out=selb[:],
                        in0=iota_rep[:, : Tb * P].rearrange("p (t o) -> p t o", o=P),
                        in1=dlb,
                        op=mybir.AluOpType.is_equal,
                    )
                    wvb = (wv_sb[:, t0 : t0 + Tb]
                           .rearrange("p (t o) -> p t o", o=1)
                           .to_broadcast([P, Tb, P]))
                    nc.vector.tensor_tensor(
                        out=selb[:], in0=selb[:], in1=wvb,
                        op=mybir.AluOpType.mult,
                    )
                    # z[d, f] = sum over tiles of selT^T @ g
                    zp = zps.tile([P, D], f32)
                    for i in range(Tb):
                        for cs in chunks:
                            nc.tensor.matmul(
                                zp[:, cs], lhsT=selb[:, i, :], rhs=g[:, i, cs],
                                start=(i == 0), stop=(i == Tb - 1),
                            )
                    # z = z_psum + dis^2 * x_self  (self loop folded in here)
                    zsb = zsbp.tile([P, D], bf16)
                    nc.vector.scalar_tensor_tensor(
                        out=zsb[:], in0=xs[:], scalar=d2_sb[:, bk : bk + 1],
                        in1=zp[:], op0=mybir.AluOpType.mult,
                        op1=mybir.AluOpType.add,
                    )
                    # transpose z via DRAM roundtrip + transposing gather
                    zd = dram.tile([P, D], bf16, tag="zd")
                    nc.sync.dma_start(zd[:], zsb[:])
                    zt = ztbp.tile([P, KT, P], bf16, tag="zt")
                    nc.gpsimd.dma_gather(
                        zt[:],
                        zd[:],
                        tix_sb[:],
                        P,
                        P,
                        D,
                        transpose=True,
                        queue_num=gq,
                        single_packet=False,
                    )
                    gq = 1 - gq
                    zts[bk] = zt
                    # software pipeline: GEMM for the previous block now, so
                    # the PE stays busy while this block's z roundtrips DRAM
                    if bk >= 1:
                        gemm_block(bk - 1)
                gemm_block(NBLK - 1)

    nc.compile()
    return nc


def kernel(x, edge_index, W, b):
    T_bs, NT, in_maps = _preprocess(x, edge_index, W, b)
    key = (T_bs, NT)
    if key not in _cache:
        _cache[key] = _build(T_bs, NT)
    nc = _cache[key]
    res = bass_utils.run_bass_kernel_spmd(nc, in_maps, core_ids=list(range(NC)))
    out = np.empty((N, D), dtype=np.float32)
    for c in range(NC):
        out[c * NPC : (c + 1) * NPC] = res.results[c]["yout"][:NPC].astype(
            np.float32
        )
    return out
